# revision 13
# baseline (speedup 1.0000x reference)
"""Trainium2 Bass kernel for the Engram module (hashed n-gram memory lookup).

Contract: kernel(**inputs) takes the FULL unsharded inputs (numpy arrays,
keyed as in setup_inputs()) and returns the FULL output (4, 2048, 2048) f32.

Sharding strategy (chosen; hardcoded):
  Pure data parallelism over tokens: 8 cores x 1024 tokens each
  (core c -> batch c//2, sequence half c%2). The 12 embedding tables are
  REPLICATED into each core's DRAM, so every gather is local and there are
  no collectives on the critical path. Gating / conv / projection weights
  are replicated too. The hash-index computation (tiny integer op count)
  is done on the host while building the per-core input maps; each core
  then gathers its own 12288 rows (256B each) from HBM with indirect DMA.

Device kernel layout (per core, window of 1152 tokens = 128 left-context
+ 1024 output tokens):
  phase A (per 128-token tile, 9 tiles):
    - indirect-DMA gather of 128*12 embedding rows -> mem [128 tok, 768] f32
    - q = hs @ W_q via PE (lhsT = host-pretransposed hs, bf16), fp32 PSUM
    - alpha = sigmoid(rowsum(q * mem)/sqrt(768)) via fused DVE
      tensor_tensor_reduce + ACT sigmoid
    - am = alpha * mem (bf16), PE-transpose into amT [768, 1152]
  phase B (per 128-hid tile, 16 tiles):
    - v^T = W_v^T @ amT on PE, hid on partitions, tokens on free dim
    - causal depthwise conv over tokens = free-dim shifted reads, scaled by
      per-partition conv weights on ACT; sums on DVE
    - fp32 residual add with host-pretransposed hs, DMA out transposed
  The host un-transposes the per-core outputs while unsharding.
"""

import numpy as np
import ml_dtypes

# ---------------- problem constants (hardcoded per the contract) -------------
B, S, HID = 4, 2048, 2048
TABLE, EMB = 200000, 64
ORDERS, HEADS = 3, 4
NSLOT = ORDERS * HEADS            # 12
MEMD = NSLOT * EMB                # 768
KCONV = 3
VOCAB = 100000
NCORES = 8
TOK = 1024                        # output tokens per core
CTX = 128                         # left context in the window
WIN = CTX + TOK                   # 1152
NTILE = WIN // 128                # 9 token tiles
ZROW = NSLOT * TABLE              # 2400000 -> the all-zeros pad row
TABROWS = ZROW + 4                # pad a few zero rows
NHID = HID // 128                 # 16 hid tiles
NMEM = MEMD // 128                # 6 mem-dim tiles
RSQRT_MEM = 1.0 / float(np.sqrt(np.float32(MEMD)))

HEAD_MULTS = np.array([2654435761, 2246822519, 3266489917, 668265263],
                      dtype=np.uint32)
POLY = np.uint32(1000003)

_BF16 = ml_dtypes.bfloat16


def _global_rows(input_ids: np.ndarray) -> np.ndarray:
    """(B, S) int -> (B, S, 12) int32 global row ids into the stacked table.

    Exactly mirrors reference.hash_indices (uint32 wraparound arithmetic),
    then offsets slot j = o*4+h by j*TABLE.
    """
    Bb, Ss = input_ids.shape
    u = input_ids.astype(np.uint32)
    per_order = []
    for n in range(2, 2 + ORDERS):
        pad = np.zeros((Bb, Ss + n - 1), np.uint32)
        pad[:, n - 1:] = u
        acc = np.zeros((Bb, Ss), np.uint32)
        for j in range(n):
            acc = acc * POLY + pad[:, j:j + Ss]
        idx = (acc[..., None] * HEAD_MULTS[None, None, :]) % np.uint32(TABLE)
        per_order.append(idx.astype(np.int32))
    gidx = np.stack(per_order, axis=2).reshape(Bb, Ss, NSLOT)
    gidx = gidx + (np.arange(NSLOT, dtype=np.int32) * TABLE)[None, None, :]
    return gidx


# ---------------- device program ---------------------------------------------
_NC_CACHE: dict = {}


def _build_nc():
    if "nc" in _NC_CACHE:
        return _NC_CACHE["nc"]

    from contextlib import ExitStack

    import concourse.bass as bass
    import concourse.mybir as mybir
    import concourse.tile as tile
    from concourse import bacc
    from concourse.masks import make_identity

    f32 = mybir.dt.float32
    bf16 = mybir.dt.bfloat16
    i32 = mybir.dt.int32
    MULT = mybir.AluOpType.mult
    ADD = mybir.AluOpType.add
    AF = mybir.ActivationFunctionType
    AXF = mybir.AxisListType

    nc = bacc.Bacc("TRN2", target_bir_lowering=False, debug=False,
                   enable_asserts=False, num_devices=NCORES)

    tab = nc.dram_tensor("tab", [TABROWS, EMB], f32, kind="ExternalInput").ap()
    hst = nc.dram_tensor("hst_bf16", [HID, WIN], bf16, kind="ExternalInput").ap()
    hsr = nc.dram_tensor("hs_res", [HID, TOK], f32, kind="ExternalInput").ap()
    wq = nc.dram_tensor("wq_bf16", [HID, MEMD], bf16, kind="ExternalInput").ap()
    wv = nc.dram_tensor("wv_bf16", [MEMD, HID], bf16, kind="ExternalInput").ap()
    idxs = nc.dram_tensor("idxs", [128, NTILE * NSLOT], i32,
                          kind="ExternalInput").ap()
    cw = nc.dram_tensor("cw", [HID, KCONV], f32, kind="ExternalInput").ap()
    cb = nc.dram_tensor("cb", [HID, 1], f32, kind="ExternalInput").ap()
    outT = nc.dram_tensor("outT", [HID, TOK], f32, kind="ExternalOutput").ap()

    with tile.TileContext(nc) as tc, ExitStack() as ctx:
        pool = lambda name, bufs, space="SBUF": ctx.enter_context(
            tc.tile_pool(name=name, bufs=bufs, space=space))

        p_const = pool("const", 1)
        p_hst = pool("hst", NHID)
        p_wq = pool("wq", NHID)
        p_wv = pool("wv", NMEM)
        p_cw = pool("cw", NHID)
        p_cb = pool("cb", NHID)
        p_amt = pool("amt", NMEM)
        p_idx = pool("idx", 1)
        p_mem = pool("mem", 4)
        p_scr = pool("scr", 2)
        p_dot = pool("dot", 4)
        p_alpha = pool("alpha", 2)
        p_am = pool("am", 3)
        p_ct = pool("ct", 6)
        p_hsr = pool("hsr", 3)
        p_s = pool("s", 4)
        p_out = pool("out", 3)
        p_qp = pool("qp", 2, space="PSUM")
        p_tp = pool("tp", 2, space="PSUM")
        p_pt = pool("pt", 3, space="PSUM")
        p_ptb = pool("ptb", 1, space="PSUM")

        ident = p_const.tile([128, 128], bf16)
        make_identity(nc, ident[:])

        # resident weights ---------------------------------------------------
        hst_sb = []
        for k in range(NHID):
            t = p_hst.tile([128, WIN], bf16, tag="hst", name=f"hst{k}")
            nc.sync.dma_start(t[:], hst[128 * k:128 * (k + 1), :])
            hst_sb.append(t)
        wq_sb = []
        for k in range(NHID):
            t = p_wq.tile([128, MEMD], bf16, tag="wq", name=f"wqt{k}")
            nc.sync.dma_start(t[:], wq[128 * k:128 * (k + 1), :])
            wq_sb.append(t)
        wv_sb = []
        for m in range(NMEM):
            t = p_wv.tile([128, HID], bf16, tag="wv", name=f"wvt{m}")
            nc.sync.dma_start(t[:], wv[128 * m:128 * (m + 1), :])
            wv_sb.append(t)
        cw_sb, cb_sb = [], []
        for m in range(NHID):
            t = p_cw.tile([128, KCONV], f32, tag="cw", name=f"cwt{m}")
            nc.sync.dma_start(t[:], cw[128 * m:128 * (m + 1), :])
            cw_sb.append(t)
            t2 = p_cb.tile([128, 1], f32, tag="cb", name=f"cbt{m}")
            nc.sync.dma_start(t2[:], cb[128 * m:128 * (m + 1), :])
            cb_sb.append(t2)

        amt_sb = [p_amt.tile([128, WIN], bf16, tag="amt", name=f"amt{m}") for m in range(NMEM)]

        # all hash indices in one load: [128, NTILE*NSLOT]
        idx_sb = p_idx.tile([128, NTILE * NSLOT], i32, tag="idx", name="idxall")
        nc.sync.dma_start(idx_sb[:], idxs[:, :])

        # phase A: gather + gate + transposed alpha*mem ----------------------
        import os
        _phases = os.environ.get("KPHASE", "AB")
        for i in range(NTILE if ("A" in _phases or _phases in ("G", "Q")) else 0):
            c0 = 128 * i
            mem_sb = p_mem.tile([128, MEMD], f32, tag="mem", name=f"memt{i}")
            if _phases == "Q":
                nc.vector.memset(mem_sb[:], 0.01)
            else:
                # HW indirect DMA takes ONE index per partition: 12/tile
                for j in range(NSLOT):
                    nc.gpsimd.indirect_dma_start(
                        out=mem_sb[:, EMB * j:EMB * (j + 1)],
                        out_offset=None,
                        in_=tab[:, :],
                        in_offset=bass.IndirectOffsetOnAxis(
                            ap=idx_sb[:, NSLOT * i + j:NSLOT * i + j + 1],
                            axis=0),
                    )
            if _phases == "G":
                nc.sync.dma_start(outT[128 * i:128 * (i + 1), 0:MEMD], mem_sb[:])
                continue

            prod = p_scr.tile([128, MEMD], f32, tag="scr", name=f"prod{i}")
            for n in range(2):
                qp = p_qp.tile([128, 384], f32, space="PSUM", tag="qp", name=f"qp{i}_{n}")
                for k in range(NHID):
                    nc.tensor.matmul(
                        qp[:],
                        lhsT=hst_sb[k][:, c0:c0 + 128],
                        rhs=wq_sb[k][:, 384 * n:384 * (n + 1)],
                        start=(k == 0),
                        stop=(k == NHID - 1),
                    )
                nc.vector.tensor_mul(prod[:, 384 * n:384 * (n + 1)], qp[:],
                                     mem_sb[:, 384 * n:384 * (n + 1)])
            dot = p_dot.tile([128, 1], f32, tag="dot", name=f"dot{i}")
            nc.vector.tensor_reduce(dot[:], prod[:], AXF.X, ADD)
            alpha = p_alpha.tile([128, 1], f32, tag="alpha", name=f"alpha{i}")
            nc.scalar.activation(alpha[:], dot[:], AF.Sigmoid, scale=RSQRT_MEM)
            am = p_am.tile([128, MEMD], bf16, tag="am", name=f"am{i}")
            nc.vector.tensor_scalar_mul(am[:], mem_sb[:], alpha[:])
            for m in range(NMEM):
                tp = p_tp.tile([128, 128], bf16, space="PSUM", tag="tp", name=f"tp{i}_{m}")
                nc.tensor.transpose(tp[:], am[:, 128 * m:128 * (m + 1)], ident[:])
                nc.vector.tensor_copy(out=amt_sb[m][:, c0:c0 + 128], in_=tp[:])

        # phase B: value projection + causal conv + residual -----------------
        if "A" not in _phases:
            for m in range(NMEM):
                nc.vector.memset(amt_sb[m][:], 0)
        for mt in range(NHID if "B" in _phases else 0):
            h0 = 128 * mt
            pa = []
            for n in range(2):
                pt = p_pt.tile([128, 512], f32, space="PSUM", tag="pt", name=f"pt{mt}_{n}")
                for m in range(NMEM):
                    nc.tensor.matmul(
                        pt[:],
                        lhsT=wv_sb[m][:, h0:h0 + 128],
                        rhs=amt_sb[m][:, 126 + 512 * n:638 + 512 * n],
                        start=(m == 0),
                        stop=(m == NMEM - 1),
                    )
                pa.append(pt)
            pb = p_ptb.tile([128, 2], f32, space="PSUM", tag="ptb", name=f"ptb{mt}")
            for m in range(NMEM):
                nc.tensor.matmul(
                    pb[:],
                    lhsT=wv_sb[m][:, h0:h0 + 128],
                    rhs=amt_sb[m][:, 1150:1152],
                    start=(m == 0),
                    stop=(m == NMEM - 1),
                )

            for n in range(2):
                A = pa[n]
                nxt = pa[1] if n == 0 else pb
                cw0 = cw_sb[mt][:, 0:1]
                cw1 = cw_sb[mt][:, 1:2]
                cw2 = cw_sb[mt][:, 2:3]
                a_t = p_ct.tile([128, 512], f32, tag="ct", name=f"at{mt}_{n}")
                nc.scalar.activation(a_t[:], A[:, 0:512], AF.Identity,
                                     bias=cb_sb[mt][:], scale=cw0)
                b_t = p_ct.tile([128, 512], f32, tag="ct", name=f"bt{mt}_{n}")
                nc.scalar.activation(b_t[:, 0:511], A[:, 1:512], AF.Identity,
                                     bias=0.0, scale=cw1)
                nc.scalar.activation(b_t[:, 511:512], nxt[:, 0:1], AF.Identity,
                                     bias=0.0, scale=cw1)
                c_t = p_ct.tile([128, 512], f32, tag="ct", name=f"ctt{mt}_{n}")
                nc.scalar.activation(c_t[:, 0:510], A[:, 2:512], AF.Identity,
                                     bias=0.0, scale=cw2)
                nc.scalar.activation(c_t[:, 510:512], nxt[:, 0:2], AF.Identity,
                                     bias=0.0, scale=cw2)
                hsr_t = p_hsr.tile([128, 512], f32, tag="hsr", name=f"hsrt{mt}_{n}")
                nc.sync.dma_start(hsr_t[:], hsr[h0:h0 + 128, 512 * n:512 * (n + 1)])
                s1 = p_s.tile([128, 512], f32, tag="s", name=f"s1_{mt}_{n}")
                nc.vector.tensor_add(s1[:], a_t[:], b_t[:])
                s2 = p_s.tile([128, 512], f32, tag="s", name=f"s2_{mt}_{n}")
                nc.vector.tensor_add(s2[:], c_t[:], hsr_t[:])
                o_t = p_out.tile([128, 512], f32, tag="out", name=f"ot{mt}_{n}")
                nc.vector.tensor_add(o_t[:], s1[:], s2[:])
                nc.sync.dma_start(outT[h0:h0 + 128, 512 * n:512 * (n + 1)], o_t[:])

    nc.compile()
    _NC_CACHE["nc"] = nc
    return nc


# ---------------- host-side sharding -----------------------------------------
def _make_in_maps(inputs: dict) -> list:
    hs = np.ascontiguousarray(np.asarray(inputs["hidden_states"], dtype=np.float32))
    ids = np.asarray(inputs["input_ids"])
    tabs = np.asarray(inputs["emb_tables"], dtype=np.float32)
    W_q = np.asarray(inputs["W_q"], dtype=np.float32)
    W_v = np.asarray(inputs["W_v"], dtype=np.float32)
    conv_w = np.asarray(inputs["conv_w"], dtype=np.float32)
    conv_b = np.asarray(inputs["conv_b"], dtype=np.float32)

    tab_full = np.zeros((TABROWS, EMB), dtype=np.float32)
    tab_full[:ZROW] = tabs.reshape(ZROW, EMB)
    gidx = _global_rows(ids)                              # (B, S, 12) int32

    wq_b = np.ascontiguousarray(W_q.astype(_BF16))        # (2048, 768)
    wv_b = np.ascontiguousarray(W_v.astype(_BF16))        # (768, 2048)
    cw2 = np.ascontiguousarray(conv_w.reshape(HID, KCONV))
    cb2 = np.ascontiguousarray(conv_b.reshape(HID, 1))

    in_maps = []
    for c in range(NCORES):
        b, h = divmod(c, 2)
        t0 = h * TOK
        lo = t0 - CTX
        v0 = max(0, lo)                                    # first valid token
        win_idx = np.full((WIN, NSLOT), ZROW, dtype=np.int32)
        win_idx[v0 - lo:] = gidx[b, v0:t0 + TOK]
        hsw = np.zeros((WIN, HID), dtype=np.float32)
        hsw[v0 - lo:] = hs[b, v0:t0 + TOK]
        hstT = np.ascontiguousarray(hsw.T)                 # (2048, 1152)
        in_maps.append({
            "tab": tab_full,
            "hst_bf16": np.ascontiguousarray(hstT.astype(_BF16)),
            "hs_res": np.ascontiguousarray(hs[b, t0:t0 + TOK].T),
            "wq_bf16": wq_b,
            "wv_bf16": wv_b,
            "idxs": np.ascontiguousarray(
                win_idx.reshape(NTILE, 128, NSLOT).transpose(1, 0, 2)
                .reshape(128, NTILE * NSLOT)),
            "cw": cw2,
            "cb": cb2,
        })
    return in_maps


def _run(inputs: dict, trace: bool = False, **kw):
    from concourse import bass_utils

    nc = _build_nc()
    in_maps = _make_in_maps(inputs)
    res = bass_utils.run_bass_kernel_spmd(
        nc, in_maps, core_ids=list(range(NCORES)), trace=trace, **kw)
    out = np.empty((B, S, HID), dtype=np.float32)
    for c in range(NCORES):
        b, h = divmod(c, 2)
        out[b, h * TOK:(h + 1) * TOK, :] = res.results[c]["outT"].T
    return out, res


def kernel(**inputs) -> np.ndarray:
    out, _ = _run(inputs, trace=False)
    return out
